# revision 35
# baseline (speedup 1.0000x reference)
"""Causal multi-head attention forward (B=2, T=2048, C=1024, H=16, D=64)
for 8 Trainium2 NeuronCores.

Sharding: core = (batch b, head-group hg) with b in {0,1}, hg in {0..3};
each core computes QKV projection for its 4 heads on its batch, causal
flash attention for those heads, and a partial output projection
(contraction over its 256 head-feature rows of W_o). Host sums the 4
partials per batch and adds b_o.

Schedule: a single globally balanced emission stream. The S->exp
pipeline is paced so ScalarE (the exp engine, ~82us of work) is fed
continuously: after each S quad, ~exp-width worth of consumer work
(PV / projection / o_proj) is emitted before the next quad, so the PE
never stalls on the psS write-after-read against the previous pair's
exp, and ScalarE never starves. exp outputs are buffered in a 4-deep
SBUF ring (es pool) so PV consumption can lag S production by up to 3
pairs, absorbing cross-engine jitter. Keepalive matmuls (reads of a
zero tile into a scratch PSUM bank) bridge any residual waits so the
HAM clock-gate never throttles the PE mid-kernel.

Per kb-pair (2 key blocks x 2 heads of the pair):
  - S matmuls (K=64) use auto-derived tile_position (0,0)/(64,0) so the
    4 quad matmuls run pairwise-concurrent on disjoint PE row halves.
  - S and PV are column-trimmed to the causally live range [jj:512];
    exp covers [jj0:1024] per (pair, h2) in one ScalarE instruction
    (the stale gap columns it exps are never read back).
  - the causal diagonal 128-square is masked by a DVE multiply with a
    precomputed triangular bf16 tile.
  - normalize: recip(l) on DVE -> gpsimd partition_broadcast -> DVE
    multiply, split in two emission pieces (recip+broadcast at the last
    PV, multiply ~1.1us of PE-work later) so the DVE queue is never
    head-of-line blocked on the gpsimd broadcast.

Kernel-internal layouts (per core):
  xT    [C, T]    bf16   x transposed (host-prepped)
  wqk   [C, 512]  bf16   [q cols heads0..3 | k cols heads0..3], q
                         pre-scaled by 1/sqrt(D) host-side
  wv    [C, 256]  bf16
  wo    [256, CO] bf16   W_o rows for this head group
  qkT   [512, T]  bf16   biases added at eviction (DVE, per-partition)
  S^T   [ki, qi]  PSUM   scores transposed, 2 kb blocks per tile
  expS  [ki, qi]  bf16   exp on ScalarE, 4-deep ring per h2
  yT'   [128, qi] PSUM   [ones|pad|v]^T @ expS -> row 0 = denominator,
                         rows 64..127 = unnormalized y^T
  out   [T, CO]   bf16   partial o_proj; host sums partials in fp32;
                         the last chunk's o_proj is split per head-pair
                         (half_a during hp1 attention, half_b + DVE add
                         in the epilogue)
"""

import os
import sys
from collections import deque
from contextlib import ExitStack
from dataclasses import dataclass

import numpy as np

for _p in ("/opt/trn_rl_repo",):
    if _p not in sys.path and os.path.isdir(_p):
        sys.path.insert(0, _p)

import ml_dtypes

import concourse.bass as bass
import concourse.bacc as bacc
import concourse.mybir as mybir
import concourse.tile as tile


def _install_axon_ntff_hook():
    """Provide antenv.axon_hooks (absent on this image) so bass_utils'
    trace path works; registers the ctypes NTFF hook when available."""
    import types

    if "antenv.axon_hooks" not in sys.modules:
        import antenv

        mod = types.ModuleType("antenv.axon_hooks")
        _reg = [None]
        mod.get_axon_ntff_profile_hook = lambda: _reg[0]
        mod.set_axon_ntff_profile_hook = lambda h: _reg.__setitem__(0, h)
        sys.modules["antenv.axon_hooks"] = mod
        antenv.axon_hooks = mod
    hooks = sys.modules["antenv.axon_hooks"]
    if hooks.get_axon_ntff_profile_hook() is not None:
        return
    try:
        import contextlib
        import ctypes

        lib = ctypes.CDLL("/opt/axon/libaxon_pjrt.so")
        if not hasattr(lib, "axon_start_nrt_profile"):
            return
        lib.axon_start_nrt_profile.argtypes = [
            ctypes.POINTER(ctypes.c_int64), ctypes.c_size_t]
        lib.axon_start_nrt_profile.restype = ctypes.c_int64
        lib.axon_stop_nrt_profile.argtypes = [ctypes.c_char_p]
        lib.axon_stop_nrt_profile.restype = ctypes.c_int64

        @contextlib.contextmanager
        def _hook(output_dir, device_ids):
            import jax

            jax.devices()
            if device_ids:
                ids = (ctypes.c_int64 * len(device_ids))(*device_ids)
                rc = lib.axon_start_nrt_profile(ids, len(device_ids))
            else:
                rc = lib.axon_start_nrt_profile(None, 0)
            if rc != 0:
                raise RuntimeError(f"axon_start_nrt_profile rc={rc}")
            try:
                yield
            finally:
                n = lib.axon_stop_nrt_profile(str(output_dir).encode())
                print(f"ntff profile: {n} file(s) -> {output_dir}",
                      file=sys.stderr)

        hooks.set_axon_ntff_profile_hook(_hook)
    except Exception:
        pass


try:
    _install_axon_ntff_hook()
except Exception:
    pass

BF16 = mybir.dt.bfloat16
F32 = mybir.dt.float32
AF = mybir.ActivationFunctionType
ALU = mybir.AluOpType
NPBF16 = ml_dtypes.bfloat16

P = 128


@dataclass(frozen=True)
class Cfg:
    T: int = 2048  # sequence length
    C: int = 1024  # input feature dim
    CO: int = 1024  # output feature dim (W_o cols)
    D: int = 64  # head dim
    HL: int = 4  # local heads per core (2 row-packed pairs)
    TQ: int = 512  # query-chunk size

    @property
    def CB(self):  # c blocks
        return self.C // P

    @property
    def NFB(self):  # qk f-blocks (q+k for HL heads)
        return 2 * self.HL * self.D // P

    @property
    def NQC(self):  # query chunks
        return self.T // self.TQ

    @property
    def TCB(self):  # t blocks of 128 (ki blocks / o_proj rows)
        return self.T // P

    @property
    def VG(self):  # v group width: [ones | pad | v] (v at partition 64)
        return self.D + 64


def emit_kernel(tc: tile.TileContext, cfg: Cfg, ins: dict, out_ap: bass.AP,
                ctx: ExitStack):
    nc = tc.nc
    T, C, CO, D, HL, TQ = cfg.T, cfg.C, cfg.CO, cfg.D, cfg.HL, cfg.TQ
    VG = cfg.VG
    CB, NQC, TCB = cfg.CB, cfg.NQC, cfg.TCB
    NHP = HL // 2
    assert HL == 4 and D == 64 and TQ == 512

    io = ctx.enter_context(tc.tile_pool(name="io", bufs=1))

    warm_sb = io.tile([P, TQ], BF16, name="warm_sb", tag="warm_sb")
    nc.vector.memset(warm_sb, 0.0)
    # prologue-critical transfers are spread over FOUR queue rings (sync,
    # gpsimd, vector, scalar are all idle during the input load) so the
    # first projection's data lands in ~2.5us instead of ~5us
    def ring(cb):
        return nc.sync if cb % 2 == 0 else nc.gpsimd

    def ring4(cb):
        return (nc.sync, nc.gpsimd, nc.scalar)[cb % 3]

    bbias_sb = io.tile([P, cfg.NFB], F32, name="bbias", tag="bbias")
    nc.sync.dma_start(bbias_sb, ins["bbias"][:, :])
    # custom-op library loads + exp ACT table preload, emitted BEFORE the
    # vector/scalar queues take prologue DMA work so the table load (1.3us)
    # happens while the input transfers stream
    tri_sb = io.tile([P, P], BF16, name="tri", tag="tri")
    nc.gpsimd.memset(tri_sb, 1.0)
    nc.gpsimd.affine_select(out=tri_sb, in_=tri_sb, compare_op=ALU.is_ge,
                            fill=0.0, base=0, channel_multiplier=-1,
                            pattern=[[1, P]])
    scr2 = io.tile([1, P], F32, name="scr2", tag="scr2")
    nc.vector.memset(scr2, 1.0)
    nc.vector.reciprocal_approx_fast(scr2, scr2)
    wqk_sb = [io.tile([P, 2 * HL * D], BF16, name=f"wqk{cb}", tag=f"wqk{cb}")
              for cb in range(CB)]
    xT_sb = [io.tile([P, T], BF16, name=f"xT{cb}", tag=f"xT{cb}")
             for cb in range(CB)]

    # interleave (wqk cb, x0 cb) pairs per ring so the chunk-0 projection
    # chain can consume cb blocks in landing order
    # (wqk, x0) pairs on sync+gpsimd ONLY: the chunk-0 projection's DMA
    # waits batch per-queue, so the scalar queue (which must run the
    # ~2.7us exp table load+drain first) cannot carry anything the first
    # projection depends on
    for cb in range(CB):
        ring(cb).dma_start(wqk_sb[cb], ins["wqk"][cb * P:(cb + 1) * P, :])
        ring(cb).dma_start(xT_sb[cb][:, 0:TQ],
                           ins["xT"][cb * P:(cb + 1) * P, 0:TQ])
    nc.scalar.activation(scr2, scr2, AF.Exp)  # exp table preload
    wv_sb = [io.tile([P, HL * D], BF16, name=f"wv{cb}", tag=f"wv{cb}")
             for cb in range(CB)]
    wo_sb = [io.tile([P, CO], BF16, name=f"wo{fb}", tag=f"wo{fb}")
             for fb in range(HL * D // P)]

    def dma_rest():
        # wv + x1 split over both main rings; everything later sync-only so
        # the gpsimd queue is free for the normalize partition_broadcasts
        # by the time the first chunk's normalize runs (the scalar queue is
        # NOT used: its table-load serializes ahead of any DMA it hosts and
        # the batched wait thresholds then stall every dependent consumer)
        for cb in range(CB):
            ring(cb).dma_start(wv_sb[cb], ins["wv"][cb * P:(cb + 1) * P, :])
        for cb in range(CB):
            ring(cb).dma_start(xT_sb[cb][:, TQ:2 * TQ],
                               ins["xT"][cb * P:(cb + 1) * P, TQ:2 * TQ])
        for fb in range(HL * D // P):
            nc.sync.dma_start(wo_sb[fb], ins["wo"][fb * P:(fb + 1) * P, :])
        for qc in (2, 3):
            for cb in range(CB):
                nc.sync.dma_start(xT_sb[cb][:, qc * TQ:(qc + 1) * TQ],
                                  ins["xT"][cb * P:(cb + 1) * P,
                                            qc * TQ:(qc + 1) * TQ])

    qkT_sb = [io.tile([P, T], BF16, name=f"qkT{fb}", tag=f"qkT{fb}")
              for fb in range(cfg.NFB)]
    v_all = io.tile([P, TCB * HL * VG], BF16, name="v_all", tag="v_all")
    # only the ones column (col 0 of each VG group) is ever read from the
    # non-v region; strided memset of those 64 columns is ~100x cheaper
    # than filling the whole tile
    nc.vector.memset(
        v_all.rearrange("p (g w) -> p g w", w=VG)[:, :, 0:1], 1.0)
    yT_sb = [io.tile([P, T], BF16, name=f"yT{hp}", tag=f"yT{hp}")
             for hp in range(NHP)]

    # PSUM pools: pp (proj/o_proj/keepalive, 2 banks) + psS (4 banks) +
    # psY (2 banks)
    pp = ctx.enter_context(tc.tile_pool(name="pp", bufs=2, space="PSUM"))
    psS = ctx.enter_context(tc.tile_pool(name="psS", bufs=1, space="PSUM"))
    psY = ctx.enter_context(tc.tile_pool(name="psY", bufs=1, space="PSUM"))
    esb = ctx.enter_context(tc.tile_pool(name="esb", bufs=10))
    asb = ctx.enter_context(tc.tile_pool(name="asb", bufs=3))
    osb = ctx.enter_context(tc.tile_pool(name="osb", bufs=3))

    # ---- scheduler state ----
    pe = [0.0]  # cumulative emitted PE-work estimate (ns, warm clock)

    def bump(d):
        pe[0] += d

    def mmest(cols):
        return cols / 2.4 * 1.08

    def keepalive(n=TQ):
        # clock-keeper: cheap matmul depending only on prior PE work so
        # the HAM never sees the PE idle during a cross-engine wait
        wps = pp.tile([P, TQ], F32, tag="pj", name="ps_ka")
        nc.tensor.matmul(wps[:, 0:n], warm_sb[:, 0:P], warm_sb[:, 0:n],
                         start=True, stop=True)
        bump(mmest(n))

    for _ in range(10):
        wps = pp.tile([P, TQ], F32, tag="pj", name="ps_warm")
        nc.tensor.matmul(wps, warm_sb[:, 0:P], warm_sb, start=True, stop=True)
        bump(mmest(TQ))

    # ---- work-group emitters ----
    def proj_qk_group(fb, tq):
        def emit():
            ps = pp.tile([P, TQ], F32, tag="pj", name="ps_qk")
            for cb in range(CB):
                nc.tensor.matmul(
                    ps,
                    wqk_sb[cb][:, fb * P:(fb + 1) * P],
                    xT_sb[cb][:, tq * TQ:(tq + 1) * TQ],
                    start=(cb == 0), stop=(cb == CB - 1))
            nc.vector.tensor_scalar(
                qkT_sb[fb][:, tq * TQ:(tq + 1) * TQ], ps,
                bbias_sb[:, fb:fb + 1], None, op0=ALU.add)
        return emit

    def proj_v_group(tb):
        def emit():
            psv = pp.tile([P, HL * D], F32, tag="pj", name="ps_v")
            for cb in range(CB):
                nc.tensor.matmul(
                    psv,
                    xT_sb[cb][:, tb * P:(tb + 1) * P],
                    wv_sb[cb],
                    start=(cb == 0), stop=(cb == CB - 1))
            # v group layout [ones | pad(63) | v]: PV then puts the softmax
            # denominator at PSUM partition 0 and y at partition 64.
            vdst = v_all[:, tb * HL * VG:(tb + 1) * HL * VG]
            vdst = vdst.rearrange("p (h g) -> p h g", g=VG)[:, :, 64:VG]
            nc.vector.tensor_copy(vdst, psv.rearrange("p (h d) -> p h d",
                                                      d=D))
        return emit

    drain_mode = [False]
    ev_flip = [0]

    def evict_copy(dst, src_ap):
        # in the drain phase ScalarE is idle (exp stream done): alternate
        # o_proj evictions between DVE and ScalarE so the eviction stream
        # is not serialized on one engine
        ev_flip[0] ^= 1
        if drain_mode[0] and ev_flip[0]:
            nc.scalar.activation(dst, src_ap, AF.Copy)
        else:
            nc.vector.tensor_copy(dst, src_ap)

    def oproj_group(tb, jc, box):
        def emit():
            if jc == 0:
                box.append(osb.tile([P, CO], BF16, tag="o_sb", name="o_sb"))
            o_sb = box[0]
            ops = pp.tile([P, TQ], F32, tag="pj", name="ps_o")
            for fb2 in range(HL * D // P):
                nc.tensor.matmul(
                    ops,
                    yT_sb[fb2][:, tb * P:(tb + 1) * P],
                    wo_sb[fb2][:, jc * TQ:(jc + 1) * TQ],
                    start=(fb2 == 0), stop=(fb2 == HL * D // P - 1))
            evict_copy(o_sb[:, jc * TQ:(jc + 1) * TQ], ops)
            if jc == CO // TQ - 1:
                nc.sync.dma_start(out_ap[tb * P:(tb + 1) * P, :], o_sb)
        return emit

    # last chunk's o_proj is split by head-pair: hp=0 halves run during
    # hp=1's attention; hp=1 halves + a DVE add form the epilogue
    oh_sb = {tb: io.tile([P, CO], BF16, name=f"oh{tb}", tag=f"oh{tb}")
             for tb in range(4 * (NQC - 1), TCB)}

    def oproj_half_a(tb, jc):
        def emit():
            ops = pp.tile([P, TQ], F32, tag="pj", name="ps_oa")
            nc.tensor.matmul(
                ops, yT_sb[0][:, tb * P:(tb + 1) * P],
                wo_sb[0][:, jc * TQ:(jc + 1) * TQ], start=True, stop=True)
            evict_copy(oh_sb[tb][:, jc * TQ:(jc + 1) * TQ], ops)
        return emit

    def oproj_half_b(tb, jc, box):
        def emit():
            if jc == 0:
                box.append(osb.tile([P, CO], BF16, tag="o_sb", name="o_sb"))
            o_sb = box[0]
            ops = pp.tile([P, TQ], F32, tag="pj", name="ps_ob")
            nc.tensor.matmul(
                ops, yT_sb[1][:, tb * P:(tb + 1) * P],
                wo_sb[1][:, jc * TQ:(jc + 1) * TQ], start=True, stop=True)
            nc.vector.tensor_tensor(
                o_sb[:, jc * TQ:(jc + 1) * TQ], ops,
                oh_sb[tb][:, jc * TQ:(jc + 1) * TQ], op=ALU.add)
            if jc == CO // TQ - 1:
                nc.sync.dma_start(out_ap[tb * P:(tb + 1) * P, :], o_sb)
        return emit

    # ---- projection task registry (deadline-ordered, gate-guarded) ----
    proj_tasks = {}
    for c in range(NQC):
        for fb in range(4):
            proj_tasks[('qk', fb, c)] = (proj_qk_group(fb, c), mmest(8 * TQ))
    for tb in range(TCB):
        proj_tasks[('v', tb)] = (proj_v_group(tb), mmest(8 * HL * D))
    emitted_proj = set()
    proj_order = ([('qk', fb, 0) for fb in (1, 3)]
                  + [('v', tb) for tb in range(4)])
    for c in range(1, NQC):
        proj_order += [('qk', fb, c) for fb in (0, 2, 1, 3)]
        proj_order += [('v', tb) for tb in range(4 * c, 4 * c + 4)]
    # pe-gates per x-chunk: don't emit a proj group before its x chunk's
    # DMA has plausibly landed (an early emission hard-stalls the PE)
    proj_gate = {0: 0.0, 1: 8000.0, 2: 12000.0, 3: 17000.0}

    def proj_chunk(key):
        return key[2] if key[0] == 'qk' else key[1] // 4

    def emit_proj(key):
        if key in emitted_proj:
            return
        emitted_proj.add(key)
        cl, d = proj_tasks[key]
        cl()
        bump(d)

    # ---- attention pipeline state ----
    pend = [0.0]  # pending consumer (PV + o_proj) PE-work estimate
    pv_backlog = deque()  # entries per (pair, h2)
    yps_map = {}
    norm_b_pending = []  # [gate_pe, fn]
    oproj_pool = deque()  # [gate_pe, closure, dur]
    psY_gate = [0.0, 0.0]
    normb_done = {}

    def after_norm_b(qc, hp):
        normb_done[(qc, hp)] = normb_done.get((qc, hp), 0) + 1
        if normb_done[(qc, hp)] < 2:
            return
        if qc < NQC - 1:
            if hp == 1:
                for tb in range(4 * qc, 4 * qc + 4):
                    box = []
                    for jc in range(CO // TQ):
                        oproj_pool.append(
                            [pe[0] + 2500, oproj_group(tb, jc, box),
                             mmest(2 * TQ)])
                        pend[0] += mmest(2 * TQ)
        elif hp == 0:
            for tb in range(4 * qc, 4 * qc + 4):
                for jc in range(CO // TQ):
                    oproj_pool.append(
                        [pe[0] + 2500, oproj_half_a(tb, jc), mmest(TQ)])
                    pend[0] += mmest(TQ)
        else:
            for tb in range(4 * qc, 4 * qc + 4):
                box = []
                for jc in range(CO // TQ):
                    oproj_pool.append(
                        [pe[0] + 3000, oproj_half_b(tb, jc, box),
                         mmest(TQ)])
                    pend[0] += mmest(TQ)

    def emit_norm_a(qc, hp, h2, yps):
        r = asb.tile([1, TQ], F32, tag=f"recip{h2}", name="recip")
        nc.vector.reciprocal_approx_fast(r, yps[h2][0:1, :])
        bc = asb.tile([D, TQ], F32, tag=f"bcsb{h2}", name="bc_sb")
        nc.gpsimd.partition_broadcast(bc, r)

        def norm_b():
            nc.vector.tensor_tensor(
                yT_sb[hp][h2 * D:(h2 + 1) * D, qc * TQ:(qc + 1) * TQ],
                yps[h2][64:D + 64, :], bc, op=ALU.mult)
            psY_gate[h2] = pe[0] + 800
            after_norm_b(qc, hp)
        norm_b_pending.append([pe[0] + 1100, norm_b])

    def emit_norm_b_ready():
        for e in norm_b_pending[:]:
            if pe[0] >= e[0]:
                e[1]()
                norm_b_pending.remove(e)

    def pv_gate_ok():
        t = pv_backlog[0]
        if (t['qc'], t['hp']) in yps_map:
            return True
        return pe[0] >= max(psY_gate)

    def emit_pv_one():
        t = pv_backlog.popleft()
        qc, hp, i, h2 = t['qc'], t['hp'], t['i'], t['h2']
        emit_proj(('v', 2 * i))
        emit_proj(('v', 2 * i + 1))
        # prefetch the NEXT pair's v blocks too: forcing them at first use
        # makes the PV matmul wait on a just-emitted DVE eviction (~1us PE
        # stall that starves ScalarE); one pair of lead hides it
        if 2 * i + 3 < 4 * (qc + 1):
            emit_proj(('v', 2 * i + 2))
            emit_proj(('v', 2 * i + 3))
        key = (qc, hp)
        if key not in yps_map:
            yps_map[key] = [psY.tile([P, TQ], F32, tag=f"y{h}",
                                     name=f"ps_y{h}") for h in range(2)]
        yps = yps_map[key]
        nkb = 4 * (qc + 1)
        jj = t['jj']
        for j2 in range(2):
            kb = 2 * i + j2
            h = hp * 2 + h2
            nc.tensor.matmul(
                yps[h2][:, jj[j2]:TQ],
                v_all[:, (kb * HL + h) * VG:(kb * HL + h) * VG + VG],
                t['es'][:, (2 * j2 + h2) * TQ + jj[j2]:
                        (2 * j2 + h2 + 1) * TQ],
                start=(kb == 0), stop=(kb == nkb - 1),
                skip_group_check=True)
        bump(t['dur'])
        pend[0] -= t['dur']
        if i == 2 * (qc + 1) - 1:
            emit_norm_a(qc, hp, h2, yps)

    es_cur = [None]

    def emit_s_half(qc, hp, i, j2):
        # One kb-block "half task": both h2 score matmuls (row-paired,
        # disjoint PE halves) into ONE [h2=0 | h2=1] PSUM tile, then one
        # exp over it. The next pair's j2 half only waits THIS pair's j2
        # exp (the older of the two), so ScalarE never gates the PE and
        # runs continuously; es rows land in a per-pair SBUF ring tile at
        # block (2*j2+h2)*TQ for PV to consume later.
        emit_proj(('qk', hp, qc))
        emit_proj(('qk', NHP + hp, i // 2))
        qtile, ktile = qkT_sb[hp], qkT_sb[NHP + hp]
        kb = 2 * i + j2
        jj = max(0, kb * P - qc * TQ)
        if j2 == 0:
            es_cur[0] = esb.tile([P, 4 * TQ], BF16, tag="es", name="es")
        es = es_cur[0]
        sps = psS.tile([P, 2 * TQ], F32, tag=f"sj{j2}", name=f"ps_sj{j2}")
        for h2 in range(2):
            r0, r1 = h2 * D, (h2 + 1) * D
            nc.tensor.matmul(
                sps[:, h2 * TQ + jj:(h2 + 1) * TQ],
                ktile[r0:r1, kb * P:(kb + 1) * P],
                qtile[r0:r1, qc * TQ + jj:(qc + 1) * TQ],
                start=True, stop=True)
        # gap cols [TQ:TQ+jj] hold stale exp values; they land in es cols
        # PV never reads (head of the h2=1 block before its jj trim)
        nc.scalar.activation(
            es[:, 2 * j2 * TQ + jj:2 * (j2 + 1) * TQ],
            sps[:, jj:2 * TQ], AF.Exp)
        if kb * P >= qc * TQ:  # mask diagonal 128-square per h2
            for h2 in range(2):
                dsq = es[:, (2 * j2 + h2) * TQ + jj:(2 * j2 + h2) * TQ
                         + jj + P]
                nc.vector.tensor_tensor(dsq, dsq, tri_sb, op=ALU.mult)
        sdur = (TQ - jj) / 2.4 * 1.08
        bump(sdur)
        if j2 == 1:
            jjp = [max(0, (2 * i + j) * P - qc * TQ) for j in range(2)]
            pvdur = (2 * TQ - jjp[0] - jjp[1]) / 2.4 * 1.08
            for h2 in range(2):
                pv_backlog.append(dict(qc=qc, hp=hp, i=i, h2=h2, es=es,
                                       jj=jjp, dur=pvdur))
                pend[0] += pvdur
        return 167.0 + (2 * TQ - jj) * 0.868  # exp duration estimate

    def pairs_inflight():
        return (len(pv_backlog) + 1) // 2

    def pick_consumer():
        emit_norm_b_ready()
        # ring safety first: the es ring's write-after-read means exp(n)
        # waits on PV(n-ring) having EXECUTED on the PE
        if len(pv_backlog) >= 8 and pv_gate_ok():
            emit_pv_one()
            return True
        # then keep projections draining steadily: a chunk-entry force-emit
        # of a whole qk group right before the S-half that reads it is a
        # ~2.7us ScalarE hole
        for key in proj_order:
            if key not in emitted_proj:
                if pe[0] >= proj_gate[proj_chunk(key)]:
                    emit_proj(key)
                    return True
                break
        if len(pv_backlog) >= 4 and pv_gate_ok():
            emit_pv_one()
            return True
        if oproj_pool and pe[0] >= oproj_pool[0][0]:
            _, cl, d = oproj_pool.popleft()
            cl()
            bump(d)
            pend[0] -= d
            return True
        if pv_backlog and pv_gate_ok():
            emit_pv_one()
            return True
        return False

    # ---- bootstrap: fb0/fb2 interleaved per-cb (consume DMA landing
    # order once, not twice) with both pp buffers held ----
    ps_fb = {0: pp.tile([P, TQ], F32, tag="pj", name="ps_qk"),
             2: pp.tile([P, TQ], F32, tag="pj", name="ps_qk")}
    for cb in range(CB):
        for fb in (0, 2):
            nc.tensor.matmul(
                ps_fb[fb],
                wqk_sb[cb][:, fb * P:(fb + 1) * P],
                xT_sb[cb][:, 0:TQ],
                start=(cb == 0), stop=(cb == CB - 1),
                skip_group_check=True)
    for fb in (0, 2):
        nc.vector.tensor_scalar(
            qkT_sb[fb][:, 0:TQ], ps_fb[fb],
            bbias_sb[:, fb:fb + 1], None, op0=ALU.add)
        emitted_proj.add(('qk', fb, 0))
        bump(mmest(8 * TQ))
    dma_rest()

    S_halves = [(qc, hp, i, j2) for qc in range(NQC) for hp in range(NHP)
                for i in range(2 * (qc + 1)) for j2 in range(2)]
    # act_clock models ScalarE's completion time; S-half emission targets
    # derive from it (not from pe + w) so consumer-size overshoot
    # self-corrects instead of permanently stretching the exp pipeline
    act_clock = 0.0
    act_end_j2 = [0.0, 0.0]
    next_target = 0.0
    for idx, (qc, hp, i, j2) in enumerate(S_halves):
        rem = len(S_halves) - idx
        while True:
            emit_norm_b_ready()
            if pairs_inflight() >= 8:
                if pv_backlog and pv_gate_ok():
                    emit_pv_one()
                elif not pick_consumer():
                    keepalive(256)
                continue
            if pe[0] >= next_target:
                break
            if not pick_consumer():
                keepalive(256)
        w = emit_s_half(qc, hp, i, j2)
        act_clock = max(act_clock, pe[0] + 150.0) + w
        act_end_j2[j2] = act_clock
        if idx + 1 < len(S_halves):
            next_target = act_end_j2[S_halves[idx + 1][3]] + 150.0

    # ---- drain: remaining PV tail, normalizes, o_proj epilogue ----
    drain_mode[0] = True
    while pv_backlog or norm_b_pending or oproj_pool:
        emit_norm_b_ready()
        if pv_backlog:
            if pv_gate_ok():
                emit_pv_one()
            else:
                keepalive(384)
            continue
        if oproj_pool and pe[0] >= oproj_pool[0][0]:
            _, cl, d = oproj_pool.popleft()
            cl()
            bump(d)
            pend[0] -= d
            continue
        if norm_b_pending or oproj_pool:
            keepalive(384)
            continue
        break
    for key in proj_order:
        emit_proj(key)


def build_program(cfg: Cfg, num_cores: int):
    nc = bacc.Bacc("TRN2", target_bir_lowering=False, debug=False,
                   num_devices=num_cores)
    ins = {
        "xT": nc.dram_tensor("xT", [cfg.C, cfg.T], BF16,
                             kind="ExternalInput").ap(),
        "wqk": nc.dram_tensor("wqk", [cfg.C, 2 * cfg.HL * cfg.D], BF16,
                              kind="ExternalInput").ap(),
        "wv": nc.dram_tensor("wv", [cfg.C, cfg.HL * cfg.D], BF16,
                             kind="ExternalInput").ap(),
        "wo": nc.dram_tensor("wo", [cfg.HL * cfg.D, cfg.CO], BF16,
                             kind="ExternalInput").ap(),
        "bbias": nc.dram_tensor("bbias", [P, cfg.NFB], F32,
                                kind="ExternalInput").ap(),
    }
    out_ap = nc.dram_tensor("out", [cfg.T, cfg.CO], BF16,
                            kind="ExternalOutput").ap()
    with tile.TileContext(nc) as tc:
        with ExitStack() as ctx:
            emit_kernel(tc, cfg, ins, out_ap, ctx)
    nc.compile()
    return nc


def prep_core_inputs(x_b: np.ndarray, W_qkv: np.ndarray, b_qkv: np.ndarray,
                     W_o: np.ndarray, heads, cfg: Cfg) -> dict:
    """x_b: [T, C] fp32 for this core's batch; heads: HL global head ids."""
    C, D, HL = cfg.C, cfg.D, cfg.HL
    scale = 1.0 / np.sqrt(D)
    qcols = np.concatenate([np.arange(h * D, (h + 1) * D) for h in heads])
    kcols = C + qcols
    vcols = 2 * C + qcols
    wqk = np.ascontiguousarray(
        np.concatenate([W_qkv[:, qcols] * scale, W_qkv[:, kcols]], axis=1)
    ).astype(NPBF16)
    wv = np.ascontiguousarray(W_qkv[:, vcols]).astype(NPBF16)
    wo = np.ascontiguousarray(W_o[qcols, :]).astype(NPBF16)
    bq = b_qkv[qcols].astype(np.float32)
    bk = b_qkv[kcols].astype(np.float32)
    bias_vec = np.concatenate([bq * scale, bk])
    bbias = np.ascontiguousarray(bias_vec.reshape(cfg.NFB, P).T)
    xT = np.ascontiguousarray(x_b.T).astype(NPBF16)
    return {"xT": xT, "wqk": wqk, "wv": wv, "wo": wo, "bbias": bbias}


_PROGRAM_CACHE = {}


def _get_program(cfg: Cfg, num_cores: int):
    key = (cfg, num_cores)
    if key not in _PROGRAM_CACHE:
        _PROGRAM_CACHE[key] = build_program(cfg, num_cores)
    return _PROGRAM_CACHE[key]


LAST_RESULTS = None


def kernel(x: np.ndarray, W_qkv: np.ndarray, b_qkv: np.ndarray,
           W_o: np.ndarray, b_o: np.ndarray) -> np.ndarray:
    global LAST_RESULTS
    from concourse.bass_utils import run_bass_kernel_spmd

    x = np.asarray(x, np.float32)
    W_qkv = np.asarray(W_qkv, np.float32)
    b_qkv = np.asarray(b_qkv, np.float32)
    W_o = np.asarray(W_o, np.float32)
    b_o = np.asarray(b_o, np.float32)

    B, T, C = x.shape
    H = 16
    cfg = Cfg(T=T, C=C, CO=W_o.shape[1], D=C // H, HL=4)
    n_cores = 8
    groups = H // cfg.HL  # 4 head groups
    assert B * groups == n_cores

    nc = _get_program(cfg, n_cores)

    in_maps = []
    for core in range(n_cores):
        b, hg = core // groups, core % groups
        heads = list(range(hg * cfg.HL, (hg + 1) * cfg.HL))
        in_maps.append(prep_core_inputs(x[b], W_qkv, b_qkv, W_o, heads, cfg))

    res = run_bass_kernel_spmd(nc, in_maps, core_ids=list(range(n_cores)))
    LAST_RESULTS = res

    out = np.zeros((B, T, cfg.CO), np.float32)
    for core in range(n_cores):
        out[core // groups] += np.asarray(res.results[core]["out"],
                                          dtype=np.float32)
    # softmax rows sum to 1, so the v-bias contributes b_v @ W_o to every
    # output row; fold it into the output bias on the host.
    bias_full = b_o + b_qkv[2 * C:3 * C] @ W_o
    out += bias_full[None, None, :].astype(np.float32)
    return out
